# revision 20
# baseline (speedup 1.0000x reference)
"""Joint-entropy (KDE logsumexp over 3x3 windows) Trainium2 kernel, v5.

Math: for each 3x3 window of pixel vectors v_n (C=3 channels),
  out[i,j] = log_norm - (1/9) * sum_n log(S_n),  S_n = sum_m exp(-2*||v_n-v_m||^2)
with log_norm = log(9) + 3*log(sqrt(2*pi)*0.5)  (h = 0.5, logits = -2*d2).

Sharding: 8 cores = 4 batches x 2 row-halves. Each core gets a host-padded
bf16 slab [130, 2, 3, 260] (row-major; plane 0 = x, plane 1 = x shifted one
column left) and produces a [128, 254] fp32 output slab (row 127 garbage,
dropped by the host). All window math is local; no collectives.

Pipeline (absolute-row E planes, 14 plane-slots):
    E0A[p,t,u] = E((p,u),(p,u+t+1))      t in {0,1}   rows 0..127
    E0B[p,t,u] = E((p+1,u),(p+1,u+t+1))  t in {0,1}   rows 1..128
    E1 [p,t,u] = E((p,u),(p+1,u+t-2))    t in 0..4    rows 0..127
    E2 [p,t,u] = E((p,u),(p+2,u+t-2))    t in 0..4    rows 0..126
- Stage B: per-channel parity-split 3D subs on VectorE (all operands
  4B-aligned via the host-shifted plane -> DVE 2x mode). d2 assembly is
  spread by latency class: E0A (gates the first matmuls) and E2 (gates the
  kernel tail, processed in plane-halves) stay on Vector/Scalar; E0B and
  parts of E1 go to the otherwise-idle GpSimd.
- Stage C: 72 accumulating TensorE matmuls with 0/1 shift-band
  stationaries; 9 role maps in PSUM fp32, 2 roles per bank, one
  accumulation group per bank (groups are bank-granular).
- Stage D: Ln(1 + S) per role from PSUM (self term rides the ACT affine),
  bf16 add tree on VectorE, one tensor_scalar, 128-partition out DMA.
- Square/Exp/Ln forced into one ACT table set; all DMAs are 128-partition
  patterns split across the SP and ACT HWDGE queues.
"""

import dataclasses

import ml_dtypes
import numpy as np

import concourse.bacc as bacc
import concourse.tile as tile
from concourse import mybir
from concourse.bass_utils import run_bass_kernel_spmd

F32 = mybir.dt.float32
BF16 = mybir.dt.bfloat16
AOP = mybir.AluOpType
AF = mybir.ActivationFunctionType

B = 4
C = 3
W = 256
PAD = 2
WT = W + 2 * PAD
ROWS_IN = 130  # 129 real rows + 1 pad row so every X tile is 128 partitions
ROWS_OUT = 127
WOUT = 254
LOG_NORM = float(np.log(9.0) + 3.0 * np.log(np.sqrt(2.0 * np.pi) * 0.5))

# role r = nr*3 + ncol -> (psum bank, slot). Roles 3,4 (nr=1) share a bank
# whose accumulation finishes with the E1 matmul block, so their Lns
# overlap E2 compute; the other banks finish staggered in the E2 block.
ROLE_SLOT = {
    3: (0, 0), 4: (0, 1),
    1: (1, 0), 2: (1, 1),
    6: (2, 0), 7: (2, 1),
    5: (3, 0), 8: (3, 1),
    0: (4, 0),
}


def _role_terms():
    """Per role (nr, ncol): list of 8 terms (tile_name, s, t, c0).

    Term value for window (i, j) = E<tile>[i + s, t, j + c0]."""
    out = {}
    for nr in range(3):
        for ncol in range(3):
            tl = []
            for mr in range(3):
                for mc in range(3):
                    if (mr, mc) == (nr, ncol):
                        continue
                    if mr == nr:
                        dc = abs(mc - ncol)
                        if nr <= 1:
                            tl.append(("E0A", nr, dc - 1, min(ncol, mc)))
                        else:
                            tl.append(("E0B", 1, dc - 1, min(ncol, mc)))
                    elif mr > nr:
                        a = mr - nr
                        dc = mc - ncol
                        tl.append((f"E{a}", nr if a == 1 else 0, dc + 2, ncol))
                    else:
                        a = nr - mr
                        dc = ncol - mc
                        tl.append((f"E{a}", mr if a == 1 else 0, dc + 2, mc))
            assert len(tl) == 8
            out[(nr, ncol)] = tl
    return out


def _ap(ap2, dims):
    """Rebuild a sliced AP's non-partition dims: `ap2` is a [P, w] slice
    whose offset marks the base element; `dims` = [[step_elems, count], ...]
    applied after the partition dim."""
    return dataclasses.replace(ap2, ap=[list(ap2.ap[0])] + [list(d) for d in dims])


class _one_act_table:
    """Force Square/Exp/Ln into natural_log_exp_and_others so the kernel
    needs a single ACT table load (set order/ids preserved)."""

    WANT = "natural_log_exp_and_others"
    FNS = frozenset({AF.Exp, AF.Ln, AF.Square})

    def __enter__(self):
        self._orig = bacc.get_activation_tables

        def patched(arch, _orig=self._orig):
            tabs = dict(_orig(arch))
            if self.WANT in tabs and self.FNS <= tabs[self.WANT]:
                tabs = {
                    k: (v if k == self.WANT else set(v) - self.FNS)
                    for k, v in tabs.items()
                }
            return tabs

        bacc.get_activation_tables = patched
        return self

    def __exit__(self, *exc):
        bacc.get_activation_tables = self._orig
        return False


def _build_program():
    nc = bacc.Bacc("TRN2")
    # xin[r, 0, c, w] = x padded; xin[r, 1, c, w] = same, shifted 1 col left
    xin = nc.dram_tensor("xin", (ROWS_IN, 2, C, WT), BF16, kind="ExternalInput")
    FP8 = mybir.dt.float8e4
    wsh = nc.dram_tensor("wsh", (128, 2, 128), FP8, kind="ExternalInput")
    yout = nc.dram_tensor("yout", (128, WOUT), BF16, kind="ExternalOutput")

    terms = _role_terms()

    with tile.TileContext(nc) as tc:
        with (
            tc.tile_pool(name="xp", bufs=1) as xp,
            tc.tile_pool(name="dp", bufs=1) as dp,
            tc.tile_pool(name="ep", bufs=1) as ep,
            tc.tile_pool(name="pp", bufs=1, space="PSUM") as pp,
            tc.tile_pool(name="sp", bufs=1) as sp,
        ):
            # ---- weights + inputs (HWDGE on both SP and ACT queues) ------
            WS = xp.tile([128, 2, 128], FP8, tag="wsh")
            nc.scalar.dma_start(out=WS, in_=wsh[:, :, :])
            XX = {}
            for s, eng in ((0, nc.sync), (1, nc.scalar), (2, nc.sync)):
                XX[s] = xp.tile([128, 2, C, WT], BF16, tag=f"xx{s}", name=f"xx{s}")
                eng.dma_start(out=XX[s], in_=xin[s : s + 128, :, :, :])

            # ---- PE warm-up: junk matmuls into bank 4 (re-zeroed later by
            # its real accumulation group) so HAM reaches 2.4 GHz before the
            # real stream starts ------------------------------------------
            # (emitted right after the weight DMA; they only need WS)
            # ---- stage B + C, interleaved on the PE ----------------------
            # All d2 accumulation happens on the TensorEngine: for each
            # <=512-element chunk of a unit, 3 accumulating identity-matmuls
            # sum the squared channels in PSUM; Exp reads PSUM directly.
            # Squares are split vector/scalar; subs stay on vector.
            # PE emission interleaves each unit's d2-matmuls + its role-sum
            # block so nothing queues behind later-ready work in the PE FIFO.
            E = {}
            S = [
                pp.tile([128, 2, WOUT], F32, tag=f"s{k}", name=f"s{k}")
                for k in range(5)
            ]
            JT = pp.tile([128, WOUT], F32, tag="junk")
            for _ in range(20):
                nc.tensor.matmul(
                    JT[:, :],
                    WS[:, 0, :],
                    _ap(WS[:, 0, 0:1], [[1, WOUT]]),
                    start=True,
                    stop=True,
                    skip_group_check=True,
                )
            # Build role-sum matmul descriptors. Terms of the two roles
            # sharing a PSUM bank that use the same stationary (shift s) and
            # the same E tile fuse into ONE N=508 matmul writing both role
            # slots (rhs = 2-row strided AP, out = both bank slots).
            TILEOF = {"E0A": ("E0AB", 0), "E0B": ("E0AB", 2),
                      "E1": ("E1", 0), "E2": ("E2", 0)}
            BLOCK = {"E0AB": 0, "E1": 1, "E2": 2}
            BANK_ORDER = {0: 0, 1: 1, 2: 2, 4: 3, 3: 4}  # bank3 (r8) last
            from collections import defaultdict as _dd
            mm_descs = []  # (block, bank, s, tilekey, rows=[(slot, gt, c0), ..])
            for bank in range(5):
                slots = sorted(
                    (sl, r) for r, (b, sl) in ROLE_SLOT.items() if b == bank
                )
                per = []
                for sl, r in slots:
                    g = _dd(list)
                    for tname, s, t, c0 in terms[(r // 3, r % 3)]:
                        tkey, toff = TILEOF[tname]
                        g[(tkey, s)].append((sl, toff + t, c0))
                    per.append(g)
                keys = set().union(*(p.keys() for p in per))
                for tkey, s in sorted(keys):
                    lists = [p.get((tkey, s), []) for p in per]
                    a = lists[0]
                    b_ = lists[1] if len(lists) > 1 else []
                    for ra, rb in zip(a, b_):
                        mm_descs.append((BLOCK[tkey], bank, s, tkey, [ra, rb]))
                    for row in a[len(b_):] + b_[len(a):]:
                        mm_descs.append((BLOCK[tkey], bank, s, tkey, [row]))
            mm_descs.sort(key=lambda m: (m[0], int(m[0] == 2 and max(r[1] for r in m[4]) > 2), BANK_ORDER[m[1]], m[2]))
            bank_last = {}
            for idx, m in enumerate(mm_descs):
                bank_last[m[1]] = idx
            started = set()
            emitted = [0]

            def emit_roles(blockidx):
                for idx, (blk, bank, s, tkey, rows) in enumerate(mm_descs):
                    if blk != blockidx:
                        continue
                    Eg, k = E[tkey]
                    base = Eg[0:k, rows[0][1], rows[0][2] : rows[0][2] + WOUT]
                    if len(rows) == 2:
                        stride = (rows[1][1] - rows[0][1]) * W + (
                            rows[1][2] - rows[0][2]
                        )
                        rhs = _ap(base, [[stride, 2], [1, WOUT]])
                        out = _ap(S[bank][:, 0, 0:WOUT], [[WOUT, 2], [1, WOUT]])
                    else:
                        rhs = base
                        out = S[bank][:, rows[0][0], :]
                    nc.tensor.matmul(
                        out,
                        WS[0:k, s, :],
                        rhs,
                        start=(bank not in started),
                        stop=(idx == bank_last[bank]),
                        skip_group_check=True,
                    )
                    started.add(bank)
                    emitted[0] += 1

            def subs_pair(D, pbase, P, xa, xb, c):
                """planes (pbase, pbase+1) = same-row pairs dc=1,2 via a
                negative-stride 2-plane operand (plane1@PAD, plane0@PAD+2)."""
                anchor = xa[0:P, 0, c, PAD : PAD + W]
                nc.vector.tensor_sub(
                    _ap(D[c][:, pbase, 0:W], [[W, 2], [1, W]]),
                    _ap(anchor, [[0, 2], [1, W]]),
                    _ap(xb[0:P, 1, c, PAD : PAD + W], [[-(C * WT - 2), 2], [1, W]]),
                )

            def subs_wide(D, P, xa, xb, c):
                """five planes dc=-2..2 at a row gap (xb = shifted-row tile)."""
                a1 = xa[0:P, 0, c, PAD : PAD + W].unsqueeze(1)
                nc.vector.tensor_sub(
                    _ap(D[c][:, 0, 0:W], [[2 * W, 3], [1, W]]),
                    a1.to_broadcast([P, 3, W]),
                    _ap(xb[0:P, 0, c, PAD - 2 : PAD - 2 + W], [[2, 3], [1, W]]),
                )
                nc.vector.tensor_sub(
                    _ap(D[c][:, 1, 0:W], [[2 * W, 2], [1, W]]),
                    a1.to_broadcast([P, 2, W]),
                    _ap(xb[0:P, 1, c, PAD - 2 : PAD - 2 + W], [[2, 2], [1, W]]),
                )

            def flat(tile_, P, off, n):
                return _ap(tile_[0:P, off // W, 0 : min(n, W)], [[1, n]])

            def d2_pe_exp(name, D, Eg, P, h0, h1, q_eng):
                """squares, then per 512-chunk: 3 accumulating identity
                matmuls -> PSUM d2, Exp(PSUM) -> Eg slice."""
                hn = h1 - h0
                q = []
                for c in range(C):
                    qc = dp.tile([P, hn, W], BF16, tag=f"q{c}_{name}",
                                 name=f"q{c}_{name}")
                    if q_eng[c] == "v":
                        nc.vector.tensor_mul(qc, D[c][:, h0:h1, :], D[c][:, h0:h1, :])
                    else:
                        nc.scalar.square(qc, D[c][:, h0:h1, :])
                    q.append(qc)
                total = hn * W
                for a in range(0, total, 512):
                    n = min(512, total - a)
                    d2c = pp.tile([128, 512], F32, tag="d2c", bufs=2, name=f"d2_{name}_{a}")
                    for ci, qc in enumerate(q):
                        nc.tensor.matmul(
                            d2c[:, 0:n],
                            WS[0:P, 0, :],
                            flat(qc, P, a, n),
                            start=(ci == 0),
                            stop=(ci == C - 1),
                            skip_group_check=True,
                        )
                    nc.scalar.activation(
                        flat(Eg, P, h0 * W + a, n), d2c[0:P, 0:n], AF.Exp, scale=-2.0
                    )

            def d2_v_exp(name, D, Eg, P, nb, q_eng, exp_halves):
                """squares, d2 via two vector adds, Exp per half from SBUF."""
                q = []
                for c in range(C):
                    qc = dp.tile([P, nb, W], BF16, tag=f"q{c}_{name}",
                                 name=f"q{c}_{name}")
                    if q_eng[c] == "v":
                        nc.vector.tensor_mul(qc, D[c], D[c])
                    else:
                        nc.scalar.square(qc, D[c])
                    q.append(qc)
                d2a = dp.tile([P, nb, W], BF16, tag=f"d2a_{name}")
                nc.vector.tensor_add(d2a, q[0], q[1])
                d2 = dp.tile([P, nb, W], BF16, tag=f"d2_{name}")
                nc.vector.tensor_add(d2, d2a, q[2])
                for h0, h1 in exp_halves:
                    nc.scalar.activation(
                        Eg[:, h0:h1, :], d2[:, h0:h1, :], AF.Exp, scale=-2.0
                    )

            # E0AB: planes 0,1 = E0A (rows 0..127); planes 2,3 = E0B (rows 1..128)
            D0 = [dp.tile([128, 4, W], BF16, tag=f"d_E0AB_{c}", name=f"d_E0AB_{c}")
                  for c in range(C)]
            for c in range(C):
                subs_pair(D0, 0, 128, XX[0], XX[0], c)
                subs_pair(D0, 2, 128, XX[1], XX[1], c)
            E0AB = ep.tile([128, 4, W], BF16, tag="e_E0AB")
            d2_pe_exp("E0AB", D0, E0AB, 128, 0, 4, "vss")
            E["E0AB"] = (E0AB, 128)
            emit_roles(0)

            D1 = [dp.tile([128, 5, W], BF16, tag=f"d_E1_{c}", name=f"d_E1_{c}")
                  for c in range(C)]
            for c in range(C):
                subs_wide(D1, 128, XX[0], XX[1], c)
            E1T = ep.tile([128, 5, W], BF16, tag="e_E1")
            d2_v_exp("E1", D1, E1T, 128, 5, "vss", ((0, 5),))
            E["E1"] = (E1T, 128)
            emit_roles(1)
            for _ in range(8):
                nc.tensor.matmul(
                    JT[:, :],
                    WS[:, 0, :],
                    _ap(WS[:, 0, 0:1], [[1, WOUT]]),
                    start=True,
                    stop=True,
                    skip_group_check=True,
                )

            D2 = [dp.tile([127, 5, W], BF16, tag=f"d_E2_{c}", name=f"d_E2_{c}")
                  for c in range(C)]
            for c in range(C):
                subs_wide(D2, 127, XX[0], XX[2], c)
            E2T = ep.tile([127, 5, W], BF16, tag="e_E2")
            d2_v_exp("E2", D2, E2T, 127, 5, "vvv", ((0, 3), (3, 5)))
            E["E2"] = (E2T, 127)
            emit_roles(2)

            # ---- stage D: ln per role (bank-stop order), 9-plane sum as
            # accumulating identity matmuls into recycled bank 0, one scalar
            # copy, DMA. Host applies out = -sum/9 + LOG_NORM. -------------
            LT = sp.tile([128, 9, WOUT], BF16, tag="lt")
            ln_order = [3, 4, 1, 2, 6, 7, 0, 5, 8]
            for r in ln_order:
                bank, slot = ROLE_SLOT[r]
                nc.scalar.activation(LT[:, r, :], S[bank][:, slot, :], AF.Ln, bias=1.0)
            for i, r in enumerate(ln_order):
                nc.tensor.matmul(
                    S[0][:, 0, :],
                    WS[:, 0, :],
                    LT[:, r, :],
                    start=(i == 0),
                    stop=(i == 8),
                    skip_group_check=True,
                )
            OUTT = sp.tile([128, WOUT], BF16, tag="out")
            nc.scalar.copy(OUTT, S[0][:, 0, :])
            nc.sync.dma_start(out=yout[:, :], in_=OUTT)
    if not nc.is_finalized():
        with _one_act_table():
            nc.finalize()
    return nc


_PROGRAM = None


def _get_program():
    global _PROGRAM
    if _PROGRAM is None:
        _PROGRAM = _build_program()
    return _PROGRAM


def _make_shift_weights():
    w = np.zeros((128, 2, 128), dtype=ml_dtypes.float8_e4m3)
    for s in range(2):
        for m in range(128):
            if m + s < 128:
                w[m + s, s, m] = 1.0
    return w


def _shard_inputs(x):
    x = np.asarray(x, dtype=np.float32)
    # [B, rows(257: 256 + pad row), 2(plain, col-shifted), C, WT]
    xp = np.zeros((B, 257, 2, C, WT), dtype=np.float32)
    xp[:, :256, 0, :, PAD : PAD + W] = x.transpose(0, 2, 1, 3)
    xp[:, :, 1, :, : WT - 1] = xp[:, :, 0, :, 1:]
    xp16 = xp.astype(ml_dtypes.bfloat16)
    wsh = _make_shift_weights()
    in_maps = []
    for core in range(8):
        b, half = divmod(core, 2)
        r0 = half * 127
        in_maps.append(
            {
                "xin": np.ascontiguousarray(xp16[b, r0 : r0 + ROWS_IN]),
                "wsh": wsh,
            }
        )
    return in_maps


def _gather(results):
    out = np.empty((B, 254, 254), dtype=np.float32)
    for core in range(8):
        b, half = divmod(core, 2)
        lt = np.asarray(results[core]["yout"][:127], dtype=np.float32)
        out[b, half * 127 : half * 127 + 127, :] = lt * (-1.0 / 9.0) + LOG_NORM
    return out


def kernel(x, **_unused):
    nc = _get_program()
    res = run_bass_kernel_spmd(nc, _shard_inputs(x), core_ids=list(range(8)))
    return _gather(res.results)


def kernel_traced(x):
    """Same as kernel() but returns (output, BassKernelResults) with trace."""
    nc = _get_program()
    res = run_bass_kernel_spmd(
        nc, _shard_inputs(x), core_ids=list(range(8)), trace=True
    )
    return _gather(res.results), res


# revision 21
# speedup vs baseline: 1.0689x; 1.0689x over previous
"""Joint-entropy (KDE logsumexp over 3x3 windows) Trainium2 kernel, v5.

Math: for each 3x3 window of pixel vectors v_n (C=3 channels),
  out[i,j] = log_norm - (1/9) * sum_n log(S_n),  S_n = sum_m exp(-2*||v_n-v_m||^2)
with log_norm = log(9) + 3*log(sqrt(2*pi)*0.5)  (h = 0.5, logits = -2*d2).

Sharding: 8 cores = 4 batches x 2 row-halves. Each core gets a host-padded
bf16 slab [130, 2, 3, 260] (row-major; plane 0 = x, plane 1 = x shifted one
column left) and produces a [128, 254] fp32 output slab (row 127 garbage,
dropped by the host). All window math is local; no collectives.

Pipeline (absolute-row E planes, 14 plane-slots):
    E0A[p,t,u] = E((p,u),(p,u+t+1))      t in {0,1}   rows 0..127
    E0B[p,t,u] = E((p+1,u),(p+1,u+t+1))  t in {0,1}   rows 1..128
    E1 [p,t,u] = E((p,u),(p+1,u+t-2))    t in 0..4    rows 0..127
    E2 [p,t,u] = E((p,u),(p+2,u+t-2))    t in 0..4    rows 0..126
- Stage B: per-channel parity-split 3D subs on VectorE (all operands
  4B-aligned via the host-shifted plane -> DVE 2x mode). d2 assembly is
  spread by latency class: E0A (gates the first matmuls) and E2 (gates the
  kernel tail, processed in plane-halves) stay on Vector/Scalar; E0B and
  parts of E1 go to the otherwise-idle GpSimd.
- Stage C: 72 accumulating TensorE matmuls with 0/1 shift-band
  stationaries; 9 role maps in PSUM fp32, 2 roles per bank, one
  accumulation group per bank (groups are bank-granular).
- Stage D: Ln(1 + S) per role from PSUM (self term rides the ACT affine),
  bf16 add tree on VectorE, one tensor_scalar, 128-partition out DMA.
- Square/Exp/Ln forced into one ACT table set; all DMAs are 128-partition
  patterns split across the SP and ACT HWDGE queues.
"""

import dataclasses

import ml_dtypes
import numpy as np

import concourse.bacc as bacc
import concourse.tile as tile
from concourse import mybir
from concourse.bass_utils import run_bass_kernel_spmd

F32 = mybir.dt.float32
BF16 = mybir.dt.bfloat16
AOP = mybir.AluOpType
AF = mybir.ActivationFunctionType

B = 4
C = 3
W = 256
PAD = 2
WT = W + 2 * PAD
ROWS_IN = 130  # 129 real rows + 1 pad row so every X tile is 128 partitions
ROWS_OUT = 127
WOUT = 254
LOG_NORM = float(np.log(9.0) + 3.0 * np.log(np.sqrt(2.0 * np.pi) * 0.5))

# role r = nr*3 + ncol -> (psum bank, slot). Roles 3,4 (nr=1) share a bank
# whose accumulation finishes with the E1 matmul block, so their Lns
# overlap E2 compute; the other banks finish staggered in the E2 block.
ROLE_SLOT = {
    3: (0, 0), 4: (0, 1),
    1: (1, 0), 2: (1, 1),
    6: (2, 0), 7: (2, 1),
    5: (3, 0), 8: (3, 1),
    0: (4, 0),
}


def _role_terms():
    """Per role (nr, ncol): list of 8 terms (tile_name, s, t, c0).

    Term value for window (i, j) = E<tile>[i + s, t, j + c0]."""
    out = {}
    for nr in range(3):
        for ncol in range(3):
            tl = []
            for mr in range(3):
                for mc in range(3):
                    if (mr, mc) == (nr, ncol):
                        continue
                    if mr == nr:
                        dc = abs(mc - ncol)
                        if nr <= 1:
                            tl.append(("E0A", nr, dc - 1, min(ncol, mc)))
                        else:
                            tl.append(("E0B", 1, dc - 1, min(ncol, mc)))
                    elif mr > nr:
                        a = mr - nr
                        dc = mc - ncol
                        tl.append((f"E{a}", nr if a == 1 else 0, dc + 2, ncol))
                    else:
                        a = nr - mr
                        dc = ncol - mc
                        tl.append((f"E{a}", mr if a == 1 else 0, dc + 2, mc))
            assert len(tl) == 8
            out[(nr, ncol)] = tl
    return out


def _ap(ap2, dims):
    """Rebuild a sliced AP's non-partition dims: `ap2` is a [P, w] slice
    whose offset marks the base element; `dims` = [[step_elems, count], ...]
    applied after the partition dim."""
    return dataclasses.replace(ap2, ap=[list(ap2.ap[0])] + [list(d) for d in dims])


class _one_act_table:
    """Force Square/Exp/Ln into natural_log_exp_and_others so the kernel
    needs a single ACT table load (set order/ids preserved)."""

    WANT = "natural_log_exp_and_others"
    FNS = frozenset({AF.Exp, AF.Ln, AF.Square})

    def __enter__(self):
        self._orig = bacc.get_activation_tables

        def patched(arch, _orig=self._orig):
            tabs = dict(_orig(arch))
            if self.WANT in tabs and self.FNS <= tabs[self.WANT]:
                tabs = {
                    k: (v if k == self.WANT else set(v) - self.FNS)
                    for k, v in tabs.items()
                }
            return tabs

        bacc.get_activation_tables = patched
        return self

    def __exit__(self, *exc):
        bacc.get_activation_tables = self._orig
        return False


def _build_program():
    nc = bacc.Bacc("TRN2")
    # xin[r, 0, c, w] = x padded; xin[r, 1, c, w] = same, shifted 1 col left
    xin = nc.dram_tensor("xin", (ROWS_IN, 2, C, WT), BF16, kind="ExternalInput")
    FP8 = mybir.dt.float8e4
    wsh = nc.dram_tensor("wsh", (128, 2, 128), FP8, kind="ExternalInput")
    yout = nc.dram_tensor("yout", (128, WOUT), BF16, kind="ExternalOutput")

    terms = _role_terms()

    with tile.TileContext(nc) as tc:
        with (
            tc.tile_pool(name="xp", bufs=1) as xp,
            tc.tile_pool(name="dp", bufs=1) as dp,
            tc.tile_pool(name="ep", bufs=1) as ep,
            tc.tile_pool(name="pp", bufs=1, space="PSUM") as pp,
            tc.tile_pool(name="sp", bufs=1) as sp,
        ):
            # ---- weights + inputs (HWDGE on both SP and ACT queues) ------
            WS = xp.tile([128, 2, 128], FP8, tag="wsh")
            nc.scalar.dma_start(out=WS, in_=wsh[:, :, :])
            XX = {}
            for s, eng in ((0, nc.sync), (1, nc.scalar), (2, nc.sync)):
                XX[s] = xp.tile([128, 2, C, WT], BF16, tag=f"xx{s}", name=f"xx{s}")
                eng.dma_start(out=XX[s], in_=xin[s : s + 128, :, :, :])

            # ---- PE warm-up: junk matmuls into bank 4 (re-zeroed later by
            # its real accumulation group) so HAM reaches 2.4 GHz before the
            # real stream starts ------------------------------------------
            # (emitted right after the weight DMA; they only need WS)
            # ---- stage B + C, interleaved on the PE ----------------------
            # All d2 accumulation happens on the TensorEngine: for each
            # <=512-element chunk of a unit, 3 accumulating identity-matmuls
            # sum the squared channels in PSUM; Exp reads PSUM directly.
            # Squares are split vector/scalar; subs stay on vector.
            # PE emission interleaves each unit's d2-matmuls + its role-sum
            # block so nothing queues behind later-ready work in the PE FIFO.
            E = {}
            S = [
                pp.tile([128, 2, WOUT], F32, tag=f"s{k}", name=f"s{k}")
                for k in range(5)
            ]
            JT = pp.tile([128, WOUT], F32, tag="junk")
            for _ in range(10):
                nc.tensor.matmul(
                    JT[:, :],
                    WS[:, 0, :],
                    _ap(WS[:, 0, 0:1], [[1, WOUT]]),
                    start=True,
                    stop=True,
                    skip_group_check=True,
                )
            # Build role-sum matmul descriptors. Terms of the two roles
            # sharing a PSUM bank that use the same stationary (shift s) and
            # the same E tile fuse into ONE N=508 matmul writing both role
            # slots (rhs = 2-row strided AP, out = both bank slots).
            TILEOF = {"E0A": ("E0AB", 0), "E0B": ("E0AB", 2),
                      "E1": ("E1", 0), "E2": ("E2", 0)}
            BLOCK = {"E0AB": 0, "E1": 1, "E2": 2}
            BANK_ORDER = {0: 0, 1: 1, 2: 2, 4: 3, 3: 4}  # bank3 (r8) last
            from collections import defaultdict as _dd
            mm_descs = []  # (block, bank, s, tilekey, rows=[(slot, gt, c0), ..])
            for bank in range(5):
                slots = sorted(
                    (sl, r) for r, (b, sl) in ROLE_SLOT.items() if b == bank
                )
                per = []
                for sl, r in slots:
                    g = _dd(list)
                    for tname, s, t, c0 in terms[(r // 3, r % 3)]:
                        tkey, toff = TILEOF[tname]
                        g[(tkey, s)].append((sl, toff + t, c0))
                    per.append(g)
                keys = set().union(*(p.keys() for p in per))
                for tkey, s in sorted(keys):
                    lists = [p.get((tkey, s), []) for p in per]
                    a = lists[0]
                    b_ = lists[1] if len(lists) > 1 else []
                    for ra, rb in zip(a, b_):
                        mm_descs.append((BLOCK[tkey], bank, s, tkey, [ra, rb]))
                    for row in a[len(b_):] + b_[len(a):]:
                        mm_descs.append((BLOCK[tkey], bank, s, tkey, [row]))
            mm_descs.sort(key=lambda m: (m[0], BANK_ORDER[m[1]], m[2]))
            bank_last = {}
            for idx, m in enumerate(mm_descs):
                bank_last[m[1]] = idx
            started = set()
            emitted = [0]

            def emit_roles(blockidx):
                for idx, (blk, bank, s, tkey, rows) in enumerate(mm_descs):
                    if blk != blockidx:
                        continue
                    Eg, k = E[tkey]
                    base = Eg[0:k, rows[0][1], rows[0][2] : rows[0][2] + WOUT]
                    if len(rows) == 2:
                        stride = (rows[1][1] - rows[0][1]) * W + (
                            rows[1][2] - rows[0][2]
                        )
                        rhs = _ap(base, [[stride, 2], [1, WOUT]])
                        out = _ap(S[bank][:, 0, 0:WOUT], [[WOUT, 2], [1, WOUT]])
                    else:
                        rhs = base
                        out = S[bank][:, rows[0][0], :]
                    nc.tensor.matmul(
                        out,
                        WS[0:k, s, :],
                        rhs,
                        start=(bank not in started),
                        stop=(idx == bank_last[bank]),
                        skip_group_check=True,
                    )
                    started.add(bank)
                    emitted[0] += 1

            def subs_pair(D, pbase, P, xa, xb, c):
                """planes (pbase, pbase+1) = same-row pairs dc=1,2 via a
                negative-stride 2-plane operand (plane1@PAD, plane0@PAD+2)."""
                anchor = xa[0:P, 0, c, PAD : PAD + W]
                nc.vector.tensor_sub(
                    _ap(D[c][:, pbase, 0:W], [[W, 2], [1, W]]),
                    _ap(anchor, [[0, 2], [1, W]]),
                    _ap(xb[0:P, 1, c, PAD : PAD + W], [[-(C * WT - 2), 2], [1, W]]),
                )

            def subs_wide(D, P, xa, xb, c):
                """five planes dc=-2..2 at a row gap (xb = shifted-row tile)."""
                a1 = xa[0:P, 0, c, PAD : PAD + W].unsqueeze(1)
                nc.vector.tensor_sub(
                    _ap(D[c][:, 0, 0:W], [[2 * W, 3], [1, W]]),
                    a1.to_broadcast([P, 3, W]),
                    _ap(xb[0:P, 0, c, PAD - 2 : PAD - 2 + W], [[2, 3], [1, W]]),
                )
                nc.vector.tensor_sub(
                    _ap(D[c][:, 1, 0:W], [[2 * W, 2], [1, W]]),
                    a1.to_broadcast([P, 2, W]),
                    _ap(xb[0:P, 1, c, PAD - 2 : PAD - 2 + W], [[2, 2], [1, W]]),
                )

            def flat(tile_, P, off, n):
                return _ap(tile_[0:P, off // W, 0 : min(n, W)], [[1, n]])

            def d2_pe_exp(name, D, Eg, P, h0, h1, q_eng):
                """squares, then per 512-chunk: 3 accumulating identity
                matmuls -> PSUM d2, Exp(PSUM) -> Eg slice."""
                hn = h1 - h0
                q = []
                for c in range(C):
                    qc = dp.tile([P, hn, W], BF16, tag=f"q{c}_{name}",
                                 name=f"q{c}_{name}")
                    if q_eng[c] == "v":
                        nc.vector.tensor_mul(qc, D[c][:, h0:h1, :], D[c][:, h0:h1, :])
                    else:
                        nc.scalar.square(qc, D[c][:, h0:h1, :])
                    q.append(qc)
                total = hn * W
                for a in range(0, total, 512):
                    n = min(512, total - a)
                    d2c = pp.tile([128, 512], F32, tag="d2c", bufs=2, name=f"d2_{name}_{a}")
                    for ci, qc in enumerate(q):
                        nc.tensor.matmul(
                            d2c[:, 0:n],
                            WS[0:P, 0, :],
                            flat(qc, P, a, n),
                            start=(ci == 0),
                            stop=(ci == C - 1),
                            skip_group_check=True,
                        )
                    nc.scalar.activation(
                        flat(Eg, P, h0 * W + a, n), d2c[0:P, 0:n], AF.Exp, scale=-2.0
                    )

            def d2_v_exp(name, D, Eg, P, nb, q_eng, exp_halves):
                """squares, d2 via two vector adds, Exp per half from SBUF."""
                q = []
                for c in range(C):
                    qc = dp.tile([P, nb, W], BF16, tag=f"q{c}_{name}",
                                 name=f"q{c}_{name}")
                    if q_eng[c] == "v":
                        nc.vector.tensor_mul(qc, D[c], D[c])
                    else:
                        nc.scalar.square(qc, D[c])
                    q.append(qc)
                d2a = dp.tile([P, nb, W], BF16, tag=f"d2a_{name}")
                nc.vector.tensor_add(d2a, q[0], q[1])
                d2 = dp.tile([P, nb, W], BF16, tag=f"d2_{name}")
                nc.vector.tensor_add(d2, d2a, q[2])
                for h0, h1 in exp_halves:
                    nc.scalar.activation(
                        Eg[:, h0:h1, :], d2[:, h0:h1, :], AF.Exp, scale=-2.0
                    )

            # E0AB: planes 0,1 = E0A (rows 0..127); planes 2,3 = E0B (rows 1..128)
            D0 = [dp.tile([128, 4, W], BF16, tag=f"d_E0AB_{c}", name=f"d_E0AB_{c}")
                  for c in range(C)]
            for c in range(C):
                subs_pair(D0, 0, 128, XX[0], XX[0], c)
                subs_pair(D0, 2, 128, XX[1], XX[1], c)
            E0AB = ep.tile([128, 4, W], BF16, tag="e_E0AB")
            d2_pe_exp("E0AB", D0, E0AB, 128, 0, 4, "vss")
            E["E0AB"] = (E0AB, 128)
            emit_roles(0)

            D1 = [dp.tile([128, 5, W], BF16, tag=f"d_E1_{c}", name=f"d_E1_{c}")
                  for c in range(C)]
            for c in range(C):
                subs_wide(D1, 128, XX[0], XX[1], c)
            E1T = ep.tile([128, 5, W], BF16, tag="e_E1")
            d2_v_exp("E1", D1, E1T, 128, 5, "vvs", ((0, 5),))
            E["E1"] = (E1T, 128)
            emit_roles(1)

            D2 = [dp.tile([127, 5, W], BF16, tag=f"d_E2_{c}", name=f"d_E2_{c}")
                  for c in range(C)]
            for c in range(C):
                subs_wide(D2, 127, XX[0], XX[2], c)
            E2T = ep.tile([127, 5, W], BF16, tag="e_E2")
            d2_v_exp("E2", D2, E2T, 127, 5, "vvs", ((0, 3), (3, 5)))
            E["E2"] = (E2T, 127)
            emit_roles(2)

            # ---- stage D: ln per role (bank-stop order), 9-plane sum as
            # accumulating identity matmuls into recycled bank 0, one scalar
            # copy, DMA. Host applies out = -sum/9 + LOG_NORM. -------------
            LT = sp.tile([128, 9, WOUT], BF16, tag="lt")
            ln_order = [3, 4, 1, 2, 6, 7, 0, 5, 8]
            for r in ln_order:
                bank, slot = ROLE_SLOT[r]
                nc.scalar.activation(LT[:, r, :], S[bank][:, slot, :], AF.Ln, bias=1.0)
            for i, r in enumerate(ln_order):
                nc.tensor.matmul(
                    S[0][:, 0, :],
                    WS[:, 0, :],
                    LT[:, r, :],
                    start=(i == 0),
                    stop=(i == 8),
                    skip_group_check=True,
                )
            OUTT = sp.tile([128, WOUT], BF16, tag="out")
            nc.scalar.copy(OUTT, S[0][:, 0, :])
            nc.sync.dma_start(out=yout[:, :], in_=OUTT)
    if not nc.is_finalized():
        with _one_act_table():
            nc.finalize()
    return nc


_PROGRAM = None


def _get_program():
    global _PROGRAM
    if _PROGRAM is None:
        _PROGRAM = _build_program()
    return _PROGRAM


def _make_shift_weights():
    w = np.zeros((128, 2, 128), dtype=ml_dtypes.float8_e4m3)
    for s in range(2):
        for m in range(128):
            if m + s < 128:
                w[m + s, s, m] = 1.0
    return w


def _shard_inputs(x):
    x = np.asarray(x, dtype=np.float32)
    # [B, rows(257: 256 + pad row), 2(plain, col-shifted), C, WT]
    xp = np.zeros((B, 257, 2, C, WT), dtype=np.float32)
    xp[:, :256, 0, :, PAD : PAD + W] = x.transpose(0, 2, 1, 3)
    xp[:, :, 1, :, : WT - 1] = xp[:, :, 0, :, 1:]
    xp16 = xp.astype(ml_dtypes.bfloat16)
    wsh = _make_shift_weights()
    in_maps = []
    for core in range(8):
        b, half = divmod(core, 2)
        r0 = half * 127
        in_maps.append(
            {
                "xin": np.ascontiguousarray(xp16[b, r0 : r0 + ROWS_IN]),
                "wsh": wsh,
            }
        )
    return in_maps


def _gather(results):
    out = np.empty((B, 254, 254), dtype=np.float32)
    for core in range(8):
        b, half = divmod(core, 2)
        lt = np.asarray(results[core]["yout"][:127], dtype=np.float32)
        out[b, half * 127 : half * 127 + 127, :] = lt * (-1.0 / 9.0) + LOG_NORM
    return out


def kernel(x, **_unused):
    nc = _get_program()
    res = run_bass_kernel_spmd(nc, _shard_inputs(x), core_ids=list(range(8)))
    return _gather(res.results)


def kernel_traced(x):
    """Same as kernel() but returns (output, BassKernelResults) with trace."""
    nc = _get_program()
    res = run_bass_kernel_spmd(
        nc, _shard_inputs(x), core_ids=list(range(8)), trace=True
    )
    return _gather(res.results), res


# revision 22
# speedup vs baseline: 1.0695x; 1.0005x over previous
"""Joint-entropy (KDE logsumexp over 3x3 windows) Trainium2 kernel.

Math: for each 3x3 window of pixel vectors v_n (C=3 channels),
  out[i,j] = log_norm - (1/9) * sum_n log(S_n),  S_n = sum_m exp(-2*||v_n-v_m||^2)
with log_norm = log(9) + 3*log(sqrt(2*pi)*0.5)  (h = 0.5, logits = -2*d2).

Sharding: 8 cores = 4 batches x 2 row-halves. Each core gets a host-padded
bf16 slab [130, 2, 3, 260] (row-major; plane 0 = x, plane 1 = x shifted one
column left) and returns a [128, 254] bf16 ln-sum slab; the host applies
out = -sum/9 + log_norm and drops row 127. All window math is local.

Pairwise exp-maps are indexed by ABSOLUTE input row (14 plane-slots, vs 27
per-anchor recomputes in the naive layout):
    E0A[p,t,u] = E((p,u),(p,u+t+1))      t in {0,1}   rows 0..127
    E0B[p,t,u] = E((p+1,u),(p+1,u+t+1))  t in {0,1}   rows 1..128
    E1 [p,t,u] = E((p,u),(p+1,u+t-2))    t in 0..4    rows 0..127
    E2 [p,t,u] = E((p,u),(p+2,u+t-2))    t in 0..4    rows 0..126
E0A/E0B share one 4-plane unit (E0AB).

Stage B: channel diffs as parity-split 3D wide subs on VectorE (the
host-shifted column plane keeps every operand 4B-aligned -> DVE 2x mode;
4D APs would drop to 1x). Squares split between VectorE muls and ScalarE
Square. d2 for E0AB is accumulated on the TensorEngine (3 identity-matmul
accumulates per 512-chunk into PSUM, Exp reads PSUM); E1/E2 d2 are VectorE
adds with Exp per plane-half so the kernel tail starts early. GpSimd does
NO elementwise work (its shared SBUF port inflates concurrent DVE ops).

Stage C: per-role window sums on the TensorEngine via 0/1 shift-band
fp8 stationaries (partition-shifted reads are free). Roles are paired into
PSUM banks by matching term structure so most of the 72 terms fuse into
two-role N=508 matmuls (43 matmuls total). PSUM accumulation groups are
bank-granular: one start/stop per bank. A junk-matmul burst after the
weight DMA warms the PE HAM clock gate.

Stage D: Ln(1 + S) per role from PSUM in bank-stop order (the +1 self term
rides the ACT affine), 9-plane sum as accumulating identity matmuls into
the recycled bank 0, one ScalarE copy, 128-partition bf16 DMA out.

Square/Exp/Ln are forced into one ACT table set; all DMAs move
128-partition patterns (HWDGE only fans out across the 16 DMA engines for
those) with 3120B-contiguous rows, split across the SP and ACT queues.

Measured: 36.4-36.7 us on trn2 (baseline kernel: 108.3 us), rel err 1.7e-3.
"""

import dataclasses

import ml_dtypes
import numpy as np

import concourse.bacc as bacc
import concourse.tile as tile
from concourse import mybir
from concourse.bass_utils import run_bass_kernel_spmd

F32 = mybir.dt.float32
BF16 = mybir.dt.bfloat16
AOP = mybir.AluOpType
AF = mybir.ActivationFunctionType

B = 4
C = 3
W = 256
PAD = 2
WT = W + 2 * PAD
ROWS_IN = 130  # 129 real rows + 1 pad row so every X tile is 128 partitions
ROWS_OUT = 127
WOUT = 254
LOG_NORM = float(np.log(9.0) + 3.0 * np.log(np.sqrt(2.0 * np.pi) * 0.5))

# role r = nr*3 + ncol -> (psum bank, slot). Roles 3,4 (nr=1) share a bank
# whose accumulation finishes with the E1 matmul block, so their Lns
# overlap E2 compute; the other banks finish staggered in the E2 block.
ROLE_SLOT = {
    3: (0, 0), 4: (0, 1),
    1: (1, 0), 2: (1, 1),
    6: (2, 0), 7: (2, 1),
    5: (3, 0), 8: (3, 1),
    0: (4, 0),
}


def _role_terms():
    """Per role (nr, ncol): list of 8 terms (tile_name, s, t, c0).

    Term value for window (i, j) = E<tile>[i + s, t, j + c0]."""
    out = {}
    for nr in range(3):
        for ncol in range(3):
            tl = []
            for mr in range(3):
                for mc in range(3):
                    if (mr, mc) == (nr, ncol):
                        continue
                    if mr == nr:
                        dc = abs(mc - ncol)
                        if nr <= 1:
                            tl.append(("E0A", nr, dc - 1, min(ncol, mc)))
                        else:
                            tl.append(("E0B", 1, dc - 1, min(ncol, mc)))
                    elif mr > nr:
                        a = mr - nr
                        dc = mc - ncol
                        tl.append((f"E{a}", nr if a == 1 else 0, dc + 2, ncol))
                    else:
                        a = nr - mr
                        dc = ncol - mc
                        tl.append((f"E{a}", mr if a == 1 else 0, dc + 2, mc))
            assert len(tl) == 8
            out[(nr, ncol)] = tl
    return out


def _ap(ap2, dims):
    """Rebuild a sliced AP's non-partition dims: `ap2` is a [P, w] slice
    whose offset marks the base element; `dims` = [[step_elems, count], ...]
    applied after the partition dim."""
    return dataclasses.replace(ap2, ap=[list(ap2.ap[0])] + [list(d) for d in dims])


class _one_act_table:
    """Force Square/Exp/Ln into natural_log_exp_and_others so the kernel
    needs a single ACT table load (set order/ids preserved)."""

    WANT = "natural_log_exp_and_others"
    FNS = frozenset({AF.Exp, AF.Ln, AF.Square})

    def __enter__(self):
        self._orig = bacc.get_activation_tables

        def patched(arch, _orig=self._orig):
            tabs = dict(_orig(arch))
            if self.WANT in tabs and self.FNS <= tabs[self.WANT]:
                tabs = {
                    k: (v if k == self.WANT else set(v) - self.FNS)
                    for k, v in tabs.items()
                }
            return tabs

        bacc.get_activation_tables = patched
        return self

    def __exit__(self, *exc):
        bacc.get_activation_tables = self._orig
        return False


def _build_program():
    nc = bacc.Bacc("TRN2")
    # xin[r, 0, c, w] = x padded; xin[r, 1, c, w] = same, shifted 1 col left
    xin = nc.dram_tensor("xin", (ROWS_IN, 2, C, WT), BF16, kind="ExternalInput")
    FP8 = mybir.dt.float8e4
    wsh = nc.dram_tensor("wsh", (128, 2, 128), FP8, kind="ExternalInput")
    yout = nc.dram_tensor("yout", (128, WOUT), BF16, kind="ExternalOutput")

    terms = _role_terms()

    with tile.TileContext(nc) as tc:
        with (
            tc.tile_pool(name="xp", bufs=1) as xp,
            tc.tile_pool(name="dp", bufs=1) as dp,
            tc.tile_pool(name="ep", bufs=1) as ep,
            tc.tile_pool(name="pp", bufs=1, space="PSUM") as pp,
            tc.tile_pool(name="sp", bufs=1) as sp,
        ):
            # ---- weights + inputs (HWDGE on both SP and ACT queues) ------
            WS = xp.tile([128, 2, 128], FP8, tag="wsh")
            nc.scalar.dma_start(out=WS, in_=wsh[:, :, :])
            XX = {}
            for s, eng in ((0, nc.sync), (1, nc.scalar), (2, nc.sync)):
                XX[s] = xp.tile([128, 2, C, WT], BF16, tag=f"xx{s}", name=f"xx{s}")
                eng.dma_start(out=XX[s], in_=xin[s : s + 128, :, :, :])

            # ---- PE warm-up: junk matmuls into bank 4 (re-zeroed later by
            # its real accumulation group) so HAM reaches 2.4 GHz before the
            # real stream starts ------------------------------------------
            # (emitted right after the weight DMA; they only need WS)
            # ---- stage B + C, interleaved on the PE ----------------------
            # All d2 accumulation happens on the TensorEngine: for each
            # <=512-element chunk of a unit, 3 accumulating identity-matmuls
            # sum the squared channels in PSUM; Exp reads PSUM directly.
            # Squares are split vector/scalar; subs stay on vector.
            # PE emission interleaves each unit's d2-matmuls + its role-sum
            # block so nothing queues behind later-ready work in the PE FIFO.
            E = {}
            S = [
                pp.tile([128, 2, WOUT], F32, tag=f"s{k}", name=f"s{k}")
                for k in range(5)
            ]
            JT = pp.tile([128, WOUT], F32, tag="junk")
            for _ in range(10):
                nc.tensor.matmul(
                    JT[:, :],
                    WS[:, 0, :],
                    _ap(WS[:, 0, 0:1], [[1, WOUT]]),
                    start=True,
                    stop=True,
                    skip_group_check=True,
                )
            # Build role-sum matmul descriptors. Terms of the two roles
            # sharing a PSUM bank that use the same stationary (shift s) and
            # the same E tile fuse into ONE N=508 matmul writing both role
            # slots (rhs = 2-row strided AP, out = both bank slots).
            TILEOF = {"E0A": ("E0AB", 0), "E0B": ("E0AB", 2),
                      "E1": ("E1", 0), "E2": ("E2", 0)}
            BLOCK = {"E0AB": 0, "E1": 1, "E2": 2}
            BANK_ORDER = {0: 0, 1: 1, 2: 2, 4: 3, 3: 4}  # bank3 (r8) last
            from collections import defaultdict as _dd
            mm_descs = []  # (block, bank, s, tilekey, rows=[(slot, gt, c0), ..])
            for bank in range(5):
                slots = sorted(
                    (sl, r) for r, (b, sl) in ROLE_SLOT.items() if b == bank
                )
                per = []
                for sl, r in slots:
                    g = _dd(list)
                    for tname, s, t, c0 in terms[(r // 3, r % 3)]:
                        tkey, toff = TILEOF[tname]
                        g[(tkey, s)].append((sl, toff + t, c0))
                    per.append(g)
                keys = set().union(*(p.keys() for p in per))
                for tkey, s in sorted(keys):
                    lists = [p.get((tkey, s), []) for p in per]
                    a = lists[0]
                    b_ = lists[1] if len(lists) > 1 else []
                    for ra, rb in zip(a, b_):
                        mm_descs.append((BLOCK[tkey], bank, s, tkey, [ra, rb]))
                    for row in a[len(b_):] + b_[len(a):]:
                        mm_descs.append((BLOCK[tkey], bank, s, tkey, [row]))
            mm_descs.sort(key=lambda m: (m[0], BANK_ORDER[m[1]], m[2]))
            bank_last = {}
            for idx, m in enumerate(mm_descs):
                bank_last[m[1]] = idx
            started = set()
            emitted = [0]

            def emit_roles(blockidx):
                for idx, (blk, bank, s, tkey, rows) in enumerate(mm_descs):
                    if blk != blockidx:
                        continue
                    Eg, k = E[tkey]
                    base = Eg[0:k, rows[0][1], rows[0][2] : rows[0][2] + WOUT]
                    if len(rows) == 2:
                        stride = (rows[1][1] - rows[0][1]) * W + (
                            rows[1][2] - rows[0][2]
                        )
                        rhs = _ap(base, [[stride, 2], [1, WOUT]])
                        out = _ap(S[bank][:, 0, 0:WOUT], [[WOUT, 2], [1, WOUT]])
                    else:
                        rhs = base
                        out = S[bank][:, rows[0][0], :]
                    nc.tensor.matmul(
                        out,
                        WS[0:k, s, :],
                        rhs,
                        start=(bank not in started),
                        stop=(idx == bank_last[bank]),
                        skip_group_check=True,
                    )
                    started.add(bank)
                    emitted[0] += 1

            def subs_pair(D, pbase, P, xa, xb, c):
                """planes (pbase, pbase+1) = same-row pairs dc=1,2 via a
                negative-stride 2-plane operand (plane1@PAD, plane0@PAD+2)."""
                anchor = xa[0:P, 0, c, PAD : PAD + W]
                nc.vector.tensor_sub(
                    _ap(D[c][:, pbase, 0:W], [[W, 2], [1, W]]),
                    _ap(anchor, [[0, 2], [1, W]]),
                    _ap(xb[0:P, 1, c, PAD : PAD + W], [[-(C * WT - 2), 2], [1, W]]),
                )

            def subs_wide(D, P, xa, xb, c):
                """five planes dc=-2..2 at a row gap (xb = shifted-row tile)."""
                a1 = xa[0:P, 0, c, PAD : PAD + W].unsqueeze(1)
                nc.vector.tensor_sub(
                    _ap(D[c][:, 0, 0:W], [[2 * W, 3], [1, W]]),
                    a1.to_broadcast([P, 3, W]),
                    _ap(xb[0:P, 0, c, PAD - 2 : PAD - 2 + W], [[2, 3], [1, W]]),
                )
                nc.vector.tensor_sub(
                    _ap(D[c][:, 1, 0:W], [[2 * W, 2], [1, W]]),
                    a1.to_broadcast([P, 2, W]),
                    _ap(xb[0:P, 1, c, PAD - 2 : PAD - 2 + W], [[2, 2], [1, W]]),
                )

            def flat(tile_, P, off, n):
                return _ap(tile_[0:P, off // W, 0 : min(n, W)], [[1, n]])

            def d2_pe_exp(name, D, Eg, P, h0, h1, q_eng):
                """squares, then per 512-chunk: 3 accumulating identity
                matmuls -> PSUM d2, Exp(PSUM) -> Eg slice."""
                hn = h1 - h0
                q = []
                for c in range(C):
                    qc = dp.tile([P, hn, W], BF16, tag=f"q{c}_{name}",
                                 name=f"q{c}_{name}")
                    if q_eng[c] == "v":
                        nc.vector.tensor_mul(qc, D[c][:, h0:h1, :], D[c][:, h0:h1, :])
                    else:
                        nc.scalar.square(qc, D[c][:, h0:h1, :])
                    q.append(qc)
                total = hn * W
                for a in range(0, total, 512):
                    n = min(512, total - a)
                    d2c = pp.tile([128, 512], F32, tag="d2c", bufs=2, name=f"d2_{name}_{a}")
                    for ci, qc in enumerate(q):
                        nc.tensor.matmul(
                            d2c[:, 0:n],
                            WS[0:P, 0, :],
                            flat(qc, P, a, n),
                            start=(ci == 0),
                            stop=(ci == C - 1),
                            skip_group_check=True,
                        )
                    nc.scalar.activation(
                        flat(Eg, P, h0 * W + a, n), d2c[0:P, 0:n], AF.Exp, scale=-2.0
                    )

            def d2_v_exp(name, D, Eg, P, nb, q_eng, exp_halves):
                """squares, d2 via two vector adds, Exp per half from SBUF."""
                q = []
                for c in range(C):
                    qc = dp.tile([P, nb, W], BF16, tag=f"q{c}_{name}",
                                 name=f"q{c}_{name}")
                    if q_eng[c] == "v":
                        nc.vector.tensor_mul(qc, D[c], D[c])
                    else:
                        nc.scalar.square(qc, D[c])
                    q.append(qc)
                d2a = dp.tile([P, nb, W], BF16, tag=f"d2a_{name}")
                nc.vector.tensor_add(d2a, q[0], q[1])
                d2 = dp.tile([P, nb, W], BF16, tag=f"d2_{name}")
                nc.vector.tensor_add(d2, d2a, q[2])
                for h0, h1 in exp_halves:
                    nc.scalar.activation(
                        Eg[:, h0:h1, :], d2[:, h0:h1, :], AF.Exp, scale=-2.0
                    )

            # E0AB: planes 0,1 = E0A (rows 0..127); planes 2,3 = E0B (rows 1..128)
            D0 = [dp.tile([128, 4, W], BF16, tag=f"d_E0AB_{c}", name=f"d_E0AB_{c}")
                  for c in range(C)]
            for c in range(C):
                subs_pair(D0, 0, 128, XX[0], XX[0], c)
                subs_pair(D0, 2, 128, XX[1], XX[1], c)
            E0AB = ep.tile([128, 4, W], BF16, tag="e_E0AB")
            d2_pe_exp("E0AB", D0, E0AB, 128, 0, 4, "vss")
            E["E0AB"] = (E0AB, 128)
            emit_roles(0)

            D1 = [dp.tile([128, 5, W], BF16, tag=f"d_E1_{c}", name=f"d_E1_{c}")
                  for c in range(C)]
            for c in range(C):
                subs_wide(D1, 128, XX[0], XX[1], c)
            E1T = ep.tile([128, 5, W], BF16, tag="e_E1")
            d2_v_exp("E1", D1, E1T, 128, 5, "vvs", ((0, 5),))
            E["E1"] = (E1T, 128)
            emit_roles(1)

            D2 = [dp.tile([127, 5, W], BF16, tag=f"d_E2_{c}", name=f"d_E2_{c}")
                  for c in range(C)]
            for c in range(C):
                subs_wide(D2, 127, XX[0], XX[2], c)
            E2T = ep.tile([127, 5, W], BF16, tag="e_E2")
            d2_v_exp("E2", D2, E2T, 127, 5, "vvs", ((0, 3), (3, 5)))
            E["E2"] = (E2T, 127)
            emit_roles(2)

            # ---- stage D: ln per role (bank-stop order), 9-plane sum as
            # accumulating identity matmuls into recycled bank 0, one scalar
            # copy, DMA. Host applies out = -sum/9 + LOG_NORM. -------------
            LT = sp.tile([128, 9, WOUT], BF16, tag="lt")
            ln_order = [3, 4, 1, 2, 6, 7, 0, 5, 8]
            for r in ln_order:
                bank, slot = ROLE_SLOT[r]
                nc.scalar.activation(LT[:, r, :], S[bank][:, slot, :], AF.Ln, bias=1.0)
            for i, r in enumerate(ln_order):
                nc.tensor.matmul(
                    S[0][:, 0, :],
                    WS[:, 0, :],
                    LT[:, r, :],
                    start=(i == 0),
                    stop=(i == 8),
                    skip_group_check=True,
                )
            OUTT = sp.tile([128, WOUT], BF16, tag="out")
            nc.scalar.copy(OUTT, S[0][:, 0, :])
            nc.sync.dma_start(out=yout[:, :], in_=OUTT)
    if not nc.is_finalized():
        with _one_act_table():
            nc.finalize()
    return nc


_PROGRAM = None


def _get_program():
    global _PROGRAM
    if _PROGRAM is None:
        _PROGRAM = _build_program()
    return _PROGRAM


def _make_shift_weights():
    w = np.zeros((128, 2, 128), dtype=ml_dtypes.float8_e4m3)
    for s in range(2):
        for m in range(128):
            if m + s < 128:
                w[m + s, s, m] = 1.0
    return w


def _shard_inputs(x):
    x = np.asarray(x, dtype=np.float32)
    # [B, rows(257: 256 + pad row), 2(plain, col-shifted), C, WT]
    xp = np.zeros((B, 257, 2, C, WT), dtype=np.float32)
    xp[:, :256, 0, :, PAD : PAD + W] = x.transpose(0, 2, 1, 3)
    xp[:, :, 1, :, : WT - 1] = xp[:, :, 0, :, 1:]
    xp16 = xp.astype(ml_dtypes.bfloat16)
    wsh = _make_shift_weights()
    in_maps = []
    for core in range(8):
        b, half = divmod(core, 2)
        r0 = half * 127
        in_maps.append(
            {
                "xin": np.ascontiguousarray(xp16[b, r0 : r0 + ROWS_IN]),
                "wsh": wsh,
            }
        )
    return in_maps


def _gather(results):
    out = np.empty((B, 254, 254), dtype=np.float32)
    for core in range(8):
        b, half = divmod(core, 2)
        lt = np.asarray(results[core]["yout"][:127], dtype=np.float32)
        out[b, half * 127 : half * 127 + 127, :] = lt * (-1.0 / 9.0) + LOG_NORM
    return out


def kernel(x, **_unused):
    nc = _get_program()
    res = run_bass_kernel_spmd(nc, _shard_inputs(x), core_ids=list(range(8)))
    return _gather(res.results)


def kernel_traced(x):
    """Same as kernel() but returns (output, BassKernelResults) with trace."""
    nc = _get_program()
    res = run_bass_kernel_spmd(
        nc, _shard_inputs(x), core_ids=list(range(8)), trace=True
    )
    return _gather(res.results), res


# revision 23
# speedup vs baseline: 1.1257x; 1.0526x over previous
"""Joint-entropy (KDE logsumexp over 3x3 windows) Trainium2 kernel.

Math: for each 3x3 window of pixel vectors v_n (C=3 channels),
  out[i,j] = log_norm - (1/9) * sum_n log(S_n),  S_n = sum_m exp(-2*||v_n-v_m||^2)
with log_norm = log(9) + 3*log(sqrt(2*pi)*0.5)  (h = 0.5, logits = -2*d2).

Sharding: 8 cores = 4 batches x 2 row-halves. Each core gets a host-padded
bf16 slab [130, 2, 3, 260] (row-major; plane 0 = x, plane 1 = x shifted one
column left) and returns a [128, 254] bf16 ln-sum slab; the host applies
out = -sum/9 + log_norm and drops row 127. All window math is local.

Pairwise exp-maps are indexed by ABSOLUTE input row (14 plane-slots, vs 27
per-anchor recomputes in the naive layout):
    E0A[p,t,u] = E((p,u),(p,u+t+1))      t in {0,1}   rows 0..127
    E0B[p,t,u] = E((p+1,u),(p+1,u+t+1))  t in {0,1}   rows 1..128
    E1 [p,t,u] = E((p,u),(p+1,u+t-2))    t in 0..4    rows 0..127
    E2 [p,t,u] = E((p,u),(p+2,u+t-2))    t in 0..4    rows 0..126
E0A/E0B share one 4-plane unit (E0AB).

Stage B: channel diffs as parity-split 3D wide subs on VectorE (the
host-shifted column plane keeps every operand 4B-aligned -> DVE 2x mode;
4D APs would drop to 1x). Squares split between VectorE muls and ScalarE
Square. d2 for E0AB is accumulated on the TensorEngine (3 identity-matmul
accumulates per 512-chunk into PSUM, Exp reads PSUM); E1/E2 d2 are VectorE
adds with Exp per plane-half so the kernel tail starts early. GpSimd does
NO elementwise work (its shared SBUF port inflates concurrent DVE ops).

Stage C: per-role window sums on the TensorEngine via 0/1 shift-band
fp8 stationaries (partition-shifted reads are free). Roles are paired into
PSUM banks by matching term structure so most of the 72 terms fuse into
two-role N=508 matmuls (43 matmuls total). PSUM accumulation groups are
bank-granular: one start/stop per bank. A junk-matmul burst after the
weight DMA warms the PE HAM clock gate.

Stage D: Ln(1 + S) per role from PSUM in bank-stop order (the +1 self term
rides the ACT affine), 9-plane sum as accumulating identity matmuls into
the recycled bank 0, one ScalarE copy, 128-partition bf16 DMA out.

Square/Exp/Ln are forced into one ACT table set; all DMAs move
128-partition patterns (HWDGE only fans out across the 16 DMA engines for
those) with 3120B-contiguous rows, split across the SP and ACT queues.

Measured: 36.4-36.7 us on trn2 (baseline kernel: 108.3 us), rel err 1.7e-3.
"""

import dataclasses

import ml_dtypes
import numpy as np

import concourse.bacc as bacc
import concourse.tile as tile
from concourse import mybir
from concourse.bass_utils import run_bass_kernel_spmd

F32 = mybir.dt.float32
BF16 = mybir.dt.bfloat16
AOP = mybir.AluOpType
AF = mybir.ActivationFunctionType

B = 4
C = 3
W = 256
PAD = 2
WT = W + 2 * PAD
ROWS_IN = 130  # 129 real rows + 1 pad row so every X tile is 128 partitions
ROWS_OUT = 127
WOUT = 254
LOG_NORM = float(np.log(9.0) + 3.0 * np.log(np.sqrt(2.0 * np.pi) * 0.5))

# role r = nr*3 + ncol -> (psum bank, slot). Roles 3,4 (nr=1) share a bank
# whose accumulation finishes with the E1 matmul block, so their Lns
# overlap E2 compute; the other banks finish staggered in the E2 block.
ROLE_SLOT = {
    3: (0, 0), 4: (0, 1),
    1: (1, 0), 2: (1, 1),
    6: (2, 0), 7: (2, 1),
    5: (3, 0), 8: (3, 1),
    0: (4, 0),
}


def _role_terms():
    """Per role (nr, ncol): list of 8 terms (tile_name, s, t, c0).

    Term value for window (i, j) = E<tile>[i + s, t, j + c0]."""
    out = {}
    for nr in range(3):
        for ncol in range(3):
            tl = []
            for mr in range(3):
                for mc in range(3):
                    if (mr, mc) == (nr, ncol):
                        continue
                    if mr == nr:
                        dc = abs(mc - ncol)
                        if nr <= 1:
                            tl.append(("E0A", nr, dc - 1, min(ncol, mc)))
                        else:
                            tl.append(("E0B", 1, dc - 1, min(ncol, mc)))
                    elif mr > nr:
                        a = mr - nr
                        dc = mc - ncol
                        tl.append((f"E{a}", nr if a == 1 else 0, dc + 2, ncol))
                    else:
                        a = nr - mr
                        dc = ncol - mc
                        tl.append((f"E{a}", mr if a == 1 else 0, dc + 2, mc))
            assert len(tl) == 8
            out[(nr, ncol)] = tl
    return out


def _ap(ap2, dims):
    """Rebuild a sliced AP's non-partition dims: `ap2` is a [P, w] slice
    whose offset marks the base element; `dims` = [[step_elems, count], ...]
    applied after the partition dim."""
    return dataclasses.replace(ap2, ap=[list(ap2.ap[0])] + [list(d) for d in dims])


class _one_act_table:
    """Force Square/Exp/Ln into natural_log_exp_and_others so the kernel
    needs a single ACT table load (set order/ids preserved)."""

    WANT = "natural_log_exp_and_others"
    FNS = frozenset({AF.Exp, AF.Ln, AF.Square})

    def __enter__(self):
        self._orig = bacc.get_activation_tables

        def patched(arch, _orig=self._orig):
            tabs = dict(_orig(arch))
            if self.WANT in tabs and self.FNS <= tabs[self.WANT]:
                tabs = {
                    k: (v if k == self.WANT else set(v) - self.FNS)
                    for k, v in tabs.items()
                }
            return tabs

        bacc.get_activation_tables = patched
        return self

    def __exit__(self, *exc):
        bacc.get_activation_tables = self._orig
        return False


def _build_program():
    nc = bacc.Bacc("TRN2")
    # xin[r, 0, c, w] = x padded; xin[r, 1, c, w] = same, shifted 1 col left
    xin = nc.dram_tensor("xin", (ROWS_IN, 2, C, WT), BF16, kind="ExternalInput")
    FP8 = mybir.dt.float8e4
    wsh = nc.dram_tensor("wsh", (128, 2, 128), FP8, kind="ExternalInput")
    yout = nc.dram_tensor("yout", (128, WOUT), BF16, kind="ExternalOutput")

    terms = _role_terms()

    with tile.TileContext(nc) as tc:
        with (
            tc.tile_pool(name="xp", bufs=1) as xp,
            tc.tile_pool(name="dp", bufs=1) as dp,
            tc.tile_pool(name="ep", bufs=1) as ep,
            tc.tile_pool(name="pp", bufs=1, space="PSUM") as pp,
            tc.tile_pool(name="sp", bufs=1) as sp,
        ):
            # ---- weights + inputs (HWDGE on both SP and ACT queues) ------
            WS = xp.tile([128, 2, 128], FP8, tag="wsh")
            XX = {}
            for s in (0, 1, 2):
                XX[s] = xp.tile([128, 2, C, WT], BF16, tag=f"xx{s}", name=f"xx{s}")
            # XX0 split by plane across both HWDGE queues so the first subs
            # start ~1us earlier; weights after the first split half.
            nc.sync.dma_start(out=XX[0][:, 0, :, :], in_=xin[0:128, 0, :, :])
            nc.scalar.dma_start(out=XX[0][:, 1, :, :], in_=xin[0:128, 1, :, :])
            nc.scalar.dma_start(out=WS, in_=wsh[:, :, :])
            nc.sync.dma_start(out=XX[1], in_=xin[1:129, :, :, :])
            nc.scalar.dma_start(out=XX[2], in_=xin[2:130, :, :, :])

            # ---- PE warm-up: junk matmuls into bank 4 (re-zeroed later by
            # its real accumulation group) so HAM reaches 2.4 GHz before the
            # real stream starts ------------------------------------------
            # (emitted right after the weight DMA; they only need WS)
            # ---- stage B + C, interleaved on the PE ----------------------
            # All d2 accumulation happens on the TensorEngine: for each
            # <=512-element chunk of a unit, 3 accumulating identity-matmuls
            # sum the squared channels in PSUM; Exp reads PSUM directly.
            # Squares are split vector/scalar; subs stay on vector.
            # PE emission interleaves each unit's d2-matmuls + its role-sum
            # block so nothing queues behind later-ready work in the PE FIFO.
            E = {}
            S = [
                pp.tile([128, 2, WOUT], F32, tag=f"s{k}", name=f"s{k}")
                for k in range(5)
            ]
            JT = pp.tile([128, WOUT], F32, tag="junk")
            for _ in range(10):
                nc.tensor.matmul(
                    JT[:, :],
                    WS[:, 0, :],
                    _ap(WS[:, 0, 0:1], [[1, WOUT]]),
                    start=True,
                    stop=True,
                    skip_group_check=True,
                )
            # Build role-sum matmul descriptors. Terms of the two roles
            # sharing a PSUM bank that use the same stationary (shift s) and
            # the same E tile fuse into ONE N=508 matmul writing both role
            # slots (rhs = 2-row strided AP, out = both bank slots).
            TILEOF = {"E0A": ("E0AB", 0), "E0B": ("E0AB", 2),
                      "E1": ("E1", 0), "E2": ("E2", 0)}
            BLOCK = {"E0AB": 0, "E1": 1, "E2": 2}
            BANK_ORDER = {0: 0, 1: 1, 2: 2, 4: 3, 3: 4}  # bank3 (r8) last
            from collections import defaultdict as _dd
            mm_descs = []  # (block, bank, s, tilekey, rows=[(slot, gt, c0), ..])
            for bank in range(5):
                slots = sorted(
                    (sl, r) for r, (b, sl) in ROLE_SLOT.items() if b == bank
                )
                per = []
                for sl, r in slots:
                    g = _dd(list)
                    for tname, s, t, c0 in terms[(r // 3, r % 3)]:
                        tkey, toff = TILEOF[tname]
                        g[(tkey, s)].append((sl, toff + t, c0))
                    per.append(g)
                keys = set().union(*(p.keys() for p in per))
                for tkey, s in sorted(keys):
                    lists = [p.get((tkey, s), []) for p in per]
                    a = lists[0]
                    b_ = lists[1] if len(lists) > 1 else []
                    for ra, rb in zip(a, b_):
                        mm_descs.append((BLOCK[tkey], bank, s, tkey, [ra, rb]))
                    for row in a[len(b_):] + b_[len(a):]:
                        mm_descs.append((BLOCK[tkey], bank, s, tkey, [row]))
            mm_descs.sort(key=lambda m: (m[0], BANK_ORDER[m[1]], m[2]))
            bank_last = {}
            for idx, m in enumerate(mm_descs):
                bank_last[m[1]] = idx
            started = set()
            emitted = [0]

            def emit_roles(blockidx):
                for idx, (blk, bank, s, tkey, rows) in enumerate(mm_descs):
                    if blk != blockidx:
                        continue
                    Eg, k = E[tkey]
                    base = Eg[0:k, rows[0][1], rows[0][2] : rows[0][2] + WOUT]
                    if len(rows) == 2:
                        stride = (rows[1][1] - rows[0][1]) * W + (
                            rows[1][2] - rows[0][2]
                        )
                        rhs = _ap(base, [[stride, 2], [1, WOUT]])
                        out = _ap(S[bank][:, 0, 0:WOUT], [[WOUT, 2], [1, WOUT]])
                    else:
                        rhs = base
                        out = S[bank][:, rows[0][0], :]
                    nc.tensor.matmul(
                        out,
                        WS[0:k, s, :],
                        rhs,
                        start=(bank not in started),
                        stop=(idx == bank_last[bank]),
                        skip_group_check=True,
                    )
                    started.add(bank)
                    emitted[0] += 1

            def subs_pair(D, pbase, P, xa, xb, c):
                """planes (pbase, pbase+1) = same-row pairs dc=1,2 via a
                negative-stride 2-plane operand (plane1@PAD, plane0@PAD+2)."""
                anchor = xa[0:P, 0, c, PAD : PAD + W]
                nc.vector.tensor_sub(
                    _ap(D[c][:, pbase, 0:W], [[W, 2], [1, W]]),
                    _ap(anchor, [[0, 2], [1, W]]),
                    _ap(xb[0:P, 1, c, PAD : PAD + W], [[-(C * WT - 2), 2], [1, W]]),
                )

            def subs_wide(D, P, xa, xb, c):
                """five planes dc=-2..2 at a row gap (xb = shifted-row tile)."""
                a1 = xa[0:P, 0, c, PAD : PAD + W].unsqueeze(1)
                nc.vector.tensor_sub(
                    _ap(D[c][:, 0, 0:W], [[2 * W, 3], [1, W]]),
                    a1.to_broadcast([P, 3, W]),
                    _ap(xb[0:P, 0, c, PAD - 2 : PAD - 2 + W], [[2, 3], [1, W]]),
                )
                nc.vector.tensor_sub(
                    _ap(D[c][:, 1, 0:W], [[2 * W, 2], [1, W]]),
                    a1.to_broadcast([P, 2, W]),
                    _ap(xb[0:P, 1, c, PAD - 2 : PAD - 2 + W], [[2, 2], [1, W]]),
                )

            def flat(tile_, P, off, n):
                return _ap(tile_[0:P, off // W, 0 : min(n, W)], [[1, n]])

            def d2_pe_exp(name, D, Eg, P, h0, h1, q_eng):
                """squares, then per 512-chunk: 3 accumulating identity
                matmuls -> PSUM d2, Exp(PSUM) -> Eg slice."""
                hn = h1 - h0
                q = []
                for c in range(C):
                    qc = dp.tile([P, hn, W], BF16, tag=f"q{c}_{name}",
                                 name=f"q{c}_{name}")
                    if q_eng[c] == "v":
                        nc.vector.tensor_mul(qc, D[c][:, h0:h1, :], D[c][:, h0:h1, :])
                    else:
                        nc.scalar.square(qc, D[c][:, h0:h1, :])
                    q.append(qc)
                total = hn * W
                for a in range(0, total, 512):
                    n = min(512, total - a)
                    d2c = pp.tile([128, 512], F32, tag="d2c", bufs=2, name=f"d2_{name}_{a}")
                    for ci, qc in enumerate(q):
                        nc.tensor.matmul(
                            d2c[:, 0:n],
                            WS[0:P, 0, :],
                            flat(qc, P, a, n),
                            start=(ci == 0),
                            stop=(ci == C - 1),
                            skip_group_check=True,
                        )
                    nc.scalar.activation(
                        flat(Eg, P, h0 * W + a, n), d2c[0:P, 0:n], AF.Exp, scale=-2.0
                    )

            def d2_v_exp(name, D, Eg, P, nb, q_eng, exp_halves):
                """squares, d2 via two vector adds, Exp per half from SBUF."""
                q = []
                for c in range(C):
                    qc = dp.tile([P, nb, W], BF16, tag=f"q{c}_{name}",
                                 name=f"q{c}_{name}")
                    if q_eng[c] == "v":
                        nc.vector.tensor_mul(qc, D[c], D[c])
                    else:
                        nc.scalar.square(qc, D[c])
                    q.append(qc)
                d2a = dp.tile([P, nb, W], BF16, tag=f"d2a_{name}")
                nc.vector.tensor_add(d2a, q[0], q[1])
                d2 = dp.tile([P, nb, W], BF16, tag=f"d2_{name}")
                nc.vector.tensor_add(d2, d2a, q[2])
                for h0, h1 in exp_halves:
                    nc.scalar.activation(
                        Eg[:, h0:h1, :], d2[:, h0:h1, :], AF.Exp, scale=-2.0
                    )

            # E0AB: planes 0,1 = E0A (rows 0..127); planes 2,3 = E0B (rows 1..128)
            D0 = [dp.tile([128, 4, W], BF16, tag=f"d_E0AB_{c}", name=f"d_E0AB_{c}")
                  for c in range(C)]
            for c in range(C):
                subs_pair(D0, 0, 128, XX[0], XX[0], c)
            for c in range(C):
                subs_pair(D0, 2, 128, XX[1], XX[1], c)
            E0AB = ep.tile([128, 4, W], BF16, tag="e_E0AB")
            d2_pe_exp("E0AB", D0, E0AB, 128, 0, 4, "vss")
            E["E0AB"] = (E0AB, 128)
            emit_roles(0)

            D1 = [dp.tile([128, 5, W], BF16, tag=f"d_E1_{c}", name=f"d_E1_{c}")
                  for c in range(C)]
            for c in range(C):
                subs_wide(D1, 128, XX[0], XX[1], c)
            E1T = ep.tile([128, 5, W], BF16, tag="e_E1")
            d2_v_exp("E1", D1, E1T, 128, 5, "vvs", ((0, 5),))
            E["E1"] = (E1T, 128)
            emit_roles(1)

            D2 = [dp.tile([127, 5, W], BF16, tag=f"d_E2_{c}", name=f"d_E2_{c}")
                  for c in range(C)]
            for c in range(C):
                subs_wide(D2, 127, XX[0], XX[2], c)
            E2T = ep.tile([127, 5, W], BF16, tag="e_E2")
            d2_v_exp("E2", D2, E2T, 127, 5, "vvs", ((0, 3), (3, 5)))
            E["E2"] = (E2T, 127)
            emit_roles(2)

            # ---- stage D: ln per role (bank-stop order), 9-plane sum as
            # accumulating identity matmuls into recycled bank 0, one scalar
            # copy, DMA. Host applies out = -sum/9 + LOG_NORM. -------------
            LT = sp.tile([128, 9, WOUT], BF16, tag="lt")
            ln_order = [3, 4, 1, 2, 6, 7, 0, 5, 8]
            for r in ln_order:
                bank, slot = ROLE_SLOT[r]
                nc.scalar.activation(LT[:, r, :], S[bank][:, slot, :], AF.Ln, bias=1.0)
            for i, r in enumerate(ln_order):
                nc.tensor.matmul(
                    S[0][:, 0, :],
                    WS[:, 0, :],
                    LT[:, r, :],
                    start=(i == 0),
                    stop=(i == 8),
                    skip_group_check=True,
                )
            OUTT = sp.tile([128, WOUT], BF16, tag="out")
            nc.scalar.copy(OUTT, S[0][:, 0, :])
            nc.sync.dma_start(out=yout[:, :], in_=OUTT)
    if not nc.is_finalized():
        with _one_act_table():
            nc.finalize()
    return nc


_PROGRAM = None


def _get_program():
    global _PROGRAM
    if _PROGRAM is None:
        _PROGRAM = _build_program()
    return _PROGRAM


def _make_shift_weights():
    w = np.zeros((128, 2, 128), dtype=ml_dtypes.float8_e4m3)
    for s in range(2):
        for m in range(128):
            if m + s < 128:
                w[m + s, s, m] = 1.0
    return w


def _shard_inputs(x):
    x = np.asarray(x, dtype=np.float32)
    # [B, rows(257: 256 + pad row), 2(plain, col-shifted), C, WT]
    xp = np.zeros((B, 257, 2, C, WT), dtype=np.float32)
    xp[:, :256, 0, :, PAD : PAD + W] = x.transpose(0, 2, 1, 3)
    xp[:, :, 1, :, : WT - 1] = xp[:, :, 0, :, 1:]
    xp16 = xp.astype(ml_dtypes.bfloat16)
    wsh = _make_shift_weights()
    in_maps = []
    for core in range(8):
        b, half = divmod(core, 2)
        r0 = half * 127
        in_maps.append(
            {
                "xin": np.ascontiguousarray(xp16[b, r0 : r0 + ROWS_IN]),
                "wsh": wsh,
            }
        )
    return in_maps


def _gather(results):
    out = np.empty((B, 254, 254), dtype=np.float32)
    for core in range(8):
        b, half = divmod(core, 2)
        lt = np.asarray(results[core]["yout"][:127], dtype=np.float32)
        out[b, half * 127 : half * 127 + 127, :] = lt * (-1.0 / 9.0) + LOG_NORM
    return out


def kernel(x, **_unused):
    nc = _get_program()
    res = run_bass_kernel_spmd(nc, _shard_inputs(x), core_ids=list(range(8)))
    return _gather(res.results)


def kernel_traced(x):
    """Same as kernel() but returns (output, BassKernelResults) with trace."""
    nc = _get_program()
    res = run_bass_kernel_spmd(
        nc, _shard_inputs(x), core_ids=list(range(8)), trace=True
    )
    return _gather(res.results), res
